# revision 27
# baseline (speedup 1.0000x reference)
"""DiffWave forward pass on 8 Trainium2 NeuronCores (Bass/Tile).

Sharding: core c -> (batch b = c//2, sequence half h = c%2). Each core computes
its 8192-sample half over a window E = 8192 + 1024: the 1024-column halo is
refreshed twice (after layers 9 and 19, the dilation-cycle boundaries, where
the receptive field per 10-layer block is 1023) by a pairwise inter-core
exchange, instead of the communication-free 3072-column halo a full 30-layer
receptive field would need. Odd cores store their half TIME-MIRRORED (host
reverses their audio window and tap order of the dilated-conv weights, and
swaps the left/right edge-bias corrections), which makes the exchange fully
symmetric SPMD: every core sends local columns [E-2048, E-1024) forward,
and refreshes its margin [E-1024, E) with the partner's block read reversed.
The exchange itself is a pairwise ReduceScatter(add) through DRAM with the
send block duplicated into both slots; the receiver subtracts its own staged
copy to recover the partner's block (exact up to one f32 rounding).

Per-core layout: resident SBUF tensor xs[128, 1024+9216+1024] (float32r):
rows 0-63 = residual trunk x (deferred 1/sqrt(2) scaling folded into weights),
rows 64-127 = skip accumulator. Dilated conv = 3 accumulating float32r matmuls
(K=64, full rate at N=512) per 512-col slice reading shifted views of xs.
Gating runs as ONE packed ACT tanh per 1024-col chunk over all 128 partitions
(sigmoid(g) = (tanh(g/2)+1)/2 with the 0.5 folded into the gate-half weights
and biases), then a DMA partition-shift of the filter half and ONE in-place
DVE bf16 multiply P = tg*tf over the gate rows of sg. The (tg+1)*tf
expansion's "+tf" term is folded into the 1x1 conv instead: that conv runs as
a K=128 matmul whose lhsT carries the op weights duplicated on rows 0-63
(applied to P) and rows 64-127 (applied to tf) — matmul cost is K-independent
so this is free, and it removes the DVE +1 tensor_scalar op. The per-layer
conditioner bias (sum-of-taps dw_W @ cond + dw_b + deferred op-bias
corrections) is applied for free via the ACT bias operand, with edge-corrected
variants on the first/last dilation-width columns. In-layer in-place updates
are deferred by one chunk so neighbouring chunks read pre-update boundary
columns.
"""

import os
import sys

sys.path.insert(0, "/opt/trn_rl_repo")

import numpy as np

import concourse.bacc as bacc
import concourse.mybir as mybir
import concourse.tile as tile
from concourse.ap import AP

f32 = mybir.dt.float32
f32r = mybir.dt.float32r
bf16 = mybir.dt.bfloat16
AF = mybir.ActivationFunctionType
ALU = mybir.AluOpType

C = 64
L = 30
B = 4
T = 16384
MAX_STEPS = 200
OWN = T // 2          # 8192 owned samples per core
PAD = 512             # frozen zero pads (max dilation)
MARGIN = 1024         # halo refreshed at each 10-layer block boundary
E = OWN + MARGIN      # 9216 compute window
WBUF = PAD + E + PAD  # 10240
CH = 1024             # column chunk (2 PSUM banks)
NCH = E // CH         # 9
DILS = [2 ** (i % 10) for i in range(L)]
EXCH_AFTER = (9, 19)  # halo exchange after these layers

_CACHE = {}


def _rev_ap(t, ncols, width, nparts=128):
    """Reversed-column AP over tile t[nparts, ncols], columns [0, width) read
    backwards. Only valid for plain f32/bf16 tiles (f32r is layout-swizzled
    and mis-reads under negative stride)."""
    return AP(tensor=t[:].tensor, offset=width - 1, ap=[[ncols, nparts], [-1, width]])


# --------------------------------------------------------------------------
# device program
# --------------------------------------------------------------------------
def _build_program(dbg=False):
    nc = bacc.Bacc(
        "TRN2",
        target_bir_lowering=False,
        debug=False,
        enable_asserts=False,
        num_devices=8,
    )

    dram = {}

    def din(name, shape, dtype):
        dram[name] = nc.dram_tensor(name, list(shape), dtype, kind="ExternalInput")
        return dram[name]

    din("aud", [1, E], f32r)
    din("w3", [C, L * 3 * 128], f32r)          # dilated conv lhsT per (l, tap)
    din("opw", [128, L * 128], bf16)           # 1x1 conv lhsT per l, rows duplicated
    # per-layer ACT bias vectors (cond + dw_b + deferred op-bias corrections),
    # computed on the host (the conditioner MLP is ~1e-4 of the FLOPs)
    din("beff", [128, L], f32)
    din("beffL", [128, L], f32)                # local-left edge (drops the -d tap)
    din("beffR", [128, L], f32)                # local-right edge (drops the +d tap)
    din("inw", [1, C], f32r)
    din("inb", [C, 1], f32)
    din("skw", [128, C], f32r)                 # rows 64-127 hold sk_W.T/sqrt(30)
    din("skb", [C, 1], f32)
    din("outw", [C, 1], bf16)
    din("outb", [1, 1], f32)
    din("zeros", [C, CH], f32r)
    o_d = nc.dram_tensor("o", [1, OWN], f32, kind="ExternalOutput")
    cc_in = [
        nc.dram_tensor(f"cc_in{i}", [2 * C, MARGIN], f32, kind="Internal")
        for i in range(len(EXCH_AFTER))
    ]
    cc_out = [
        nc.dram_tensor(f"cc_out{i}", [C, MARGIN], f32, kind="Internal")
        for i in range(len(EXCH_AFTER))
    ]

    with tile.TileContext(nc) as tc:
        import contextlib

        ctx = contextlib.ExitStack()
        with ctx:
            const = ctx.enter_context(tc.tile_pool(name="const", bufs=1))
            sgp = ctx.enter_context(tc.tile_pool(name="sgp", bufs=4))
            sgfp = ctx.enter_context(tc.tile_pool(name="sgfp", bufs=4))
            hhp = ctx.enter_context(tc.tile_pool(name="hhp", bufs=2))
            otp = ctx.enter_context(tc.tile_pool(name="otp", bufs=2))
            exp = ctx.enter_context(tc.tile_pool(name="exp", bufs=2))
            dil_ps = ctx.enter_context(tc.tile_pool(name="dil_ps", bufs=2, space="PSUM"))
            op_ps = ctx.enter_context(tc.tile_pool(name="op_ps", bufs=2, space="PSUM"))

            # ---- resident state + weights ----
            xs = const.tile([128, WBUF], f32r)
            w3 = const.tile([C, L * 3 * 128], f32r)
            opw = const.tile([128, L * 128], bf16)
            beff = const.tile([128, L], f32)
            beffL = const.tile([128, L], f32)
            beffR = const.tile([128, L], f32)
            inw = const.tile([1, C], f32r)
            inb = const.tile([C, 1], f32)
            skw = const.tile([128, C], f32r)
            skb = const.tile([C, 1], f32)
            outw = const.tile([C, 1], bf16)
            outb = const.tile([1, 1], f32)

            nc.sync.dma_start(inw[:], dram["inw"].ap())
            nc.sync.dma_start(inb[:], dram["inb"].ap())
            nc.sync.dma_start(beff[:], dram["beff"].ap())
            nc.sync.dma_start(beffL[:], dram["beffL"].ap())
            nc.sync.dma_start(beffR[:], dram["beffR"].ap())

            # ---- zero pads and skip accumulator (DMA: memset-f32r fails codegen) ----
            nc.sync.dma_start(xs[0:C, 0:PAD], dram["zeros"].ap()[:, 0:PAD])
            nc.sync.dma_start(xs[0:C, PAD + E : WBUF], dram["zeros"].ap()[:, 0:PAD])
            for c in range(WBUF // CH):
                nc.sync.dma_start(
                    xs[C:128, c * CH : (c + 1) * CH], dram["zeros"].ap()
                )

            # big weights after the input-conv path so compute starts early
            nc.sync.dma_start(w3[:], dram["w3"].ap())
            nc.sync.dma_start(opw[:], dram["opw"].ap())
            nc.sync.dma_start(skw[:], dram["skw"].ap())
            nc.sync.dma_start(skb[:], dram["skb"].ap())
            nc.sync.dma_start(outw[:], dram["outw"].ap())
            nc.sync.dma_start(outb[:], dram["outb"].ap())

            with tc.tile_pool(name="audp", bufs=2) as audp:
                # ---- input conv: x0 = relu(in_W * audio + in_b) ----
                for c in range(NCH):
                    at = audp.tile([1, CH], f32r, tag="aud")
                    nc.sync.dma_start(at[:], dram["aud"].ap()[:, c * CH : (c + 1) * CH])
                    x0 = dil_ps.tile([128, CH], f32, tag="dil")
                    for s in (0, 512):
                        nc.tensor.matmul(
                            x0[0:C, s : s + 512],
                            lhsT=inw[:],
                            rhs=at[:, s : s + 512],
                            start=True,
                            stop=True,
                        )
                    nc.scalar.activation(
                        xs[0:C, PAD + c * CH : PAD + (c + 1) * CH],
                        x0[0:C, :],
                        AF.Relu,
                        bias=inb[:, 0:1],
                    )

            # ---- 30 residual layers ----
            # pend[l]: chunk c+1's tap-0 matmuls read the last d columns of
            # chunk c, so chunk c's in-place update is emitted only after chunk
            # c+1's dilated-conv reads (one-chunk lag; Tile orders by program
            # order). The emission schedule runs chunks in a wavefront around
            # each halo exchange so the engines' in-order queues never park on
            # a collective-dependent instruction (head-of-line blocking).
            pend = {}
            ex_state = {}

            def flush(l):
                if pend.get(l) is not None:
                    pcol, pop = pend[l]
                    nc.vector.tensor_add(
                        xs[:, pcol : pcol + CH], xs[:, pcol : pcol + CH], pop[:]
                    )
                    pend[l] = None

            def emit_chunk(l, c):
                d = DILS[l]
                col = PAD + c * CH
                dil = dil_ps.tile([128, CH], f32, tag="dil")
                for k in range(3):
                    off = (k - 1) * d
                    for s in (0, 512):
                        nc.tensor.matmul(
                            dil[:, s : s + 512],
                            lhsT=w3[:, (l * 3 + k) * 128 : (l * 3 + k + 1) * 128],
                            rhs=xs[0:C, col + off + s : col + off + s + 512],
                            start=(k == 0),
                            stop=(k == 2),
                        )
                sg = sgp.tile([128, CH], bf16, tag="sg")
                # (column range, bias) pieces: sequence-edge columns use the
                # tap-dropped bias (reference zero-pads x+cond, so the
                # missing tap must not contribute cond/omega via the bias)
                if c == 0:
                    pieces = [(0, d, beffL), (d, CH, beff)]
                elif c == NCH - 1:
                    pieces = [(0, CH - d, beff), (CH - d, CH, beffR)]
                else:
                    pieces = [(0, CH, beff)]
                # single packed tanh: rows 0-63 hold tanh(g/2) (gate half
                # pre-scaled 0.5 in weights; sigmoid(g) = (tanh(g/2)+1)/2),
                # rows 64-127 hold tanh(f)
                for lo, hi, bv in pieces:
                    nc.scalar.activation(
                        sg[:, lo:hi], dil[:, lo:hi], AF.Tanh,
                        bias=bv[:, l : l + 1],
                    )
                sgf = sgfp.tile([C, CH], bf16, tag="sgf")
                nc.sync.dma_start(sgf[:], sg[C:128, :])
                # P = tg*tf in place over the gate rows; the op conv below
                # contracts K=128 over [P; tf] with duplicated op weights,
                # realizing (tg+1)*tf without a separate +1 op
                nc.vector.tensor_mul(sg[0:C, :], sg[0:C, :], sgf[:])
                flush(l)
                op = op_ps.tile([128, CH], f32, tag="op")
                for s in (0, 512):
                    nc.tensor.matmul(
                        op[:, s : s + 512],
                        lhsT=opw[:, l * 128 : (l + 1) * 128],
                        rhs=sg[:, s : s + 512],
                        start=True,
                        stop=True,
                    )
                pend[l] = (col, op)

            def emit_exchange(l):
                # trunk rows only (margin skip is never consumed), f32; all
                # DMAs on the idle gpsimd (SWDGE) queue so the in-order
                # SP/DVE streams never block behind the collective
                ex = EXCH_AFTER.index(l)
                send = xs[0:C, PAD + E - 2 * MARGIN : PAD + E - MARGIN]
                nc.gpsimd.dma_start(cc_in[ex].ap()[0:C], send)
                nc.gpsimd.dma_start(cc_in[ex].ap()[C : 2 * C], send)
                nc.gpsimd.collective_compute(
                    "ReduceScatter", ALU.add,
                    replica_groups=[[0, 1], [2, 3], [4, 5], [6, 7]],
                    ins=[cc_in[ex].ap()], outs=[cc_out[ex].ap()],
                )
                rt = exp.tile([C, MARGIN], f32, tag="rt")
                nc.gpsimd.dma_start(rt[:], cc_out[ex].ap())
                # forward f32r->f32 copy of the sent block, consumed reversed
                # by the subtract (f32r cannot be read with negative stride);
                # runs any time while the collective is in flight
                stage = exp.tile([C, MARGIN], f32, tag="stage")
                nc.vector.tensor_copy(stage[:], send)
                ex_state[l] = (rt, stage)

            def emit_subtract(l):
                # margin <- reverse(partner block) = reverse(rt - stage);
                # both reads reversed, write forward into the f32r trunk
                rt, stage = ex_state.pop(l)
                nc.vector.tensor_tensor(
                    xs[0:C, PAD + E - MARGIN : PAD + E],
                    _rev_ap(rt, MARGIN, MARGIN, C),
                    _rev_ap(stage, MARGIN, MARGIN, C),
                    ALU.subtract,
                )

            def emit_tail(c):
                # skip head + output conv over one owned chunk (the margin
                # chunk's output is discarded by the host)
                col = PAD + c * CH
                hps = dil_ps.tile([128, CH], f32, tag="dil")
                for s in (0, 512):
                    nc.tensor.matmul(
                        hps[0:C, s : s + 512],
                        lhsT=skw[C:128, :],
                        rhs=xs[C:128, col + s : col + s + 512],
                        start=True,
                        stop=True,
                        tile_position=(64, 0),
                    )
                hh = hhp.tile([C, CH], bf16, tag="hh")
                nc.scalar.activation(hh[:], hps[0:C, :], AF.Relu, bias=skb[:, 0:1])
                ops2 = op_ps.tile([128, CH], f32, tag="op")
                for s in (0, 512):
                    nc.tensor.matmul(
                        ops2[0:1, s : s + 512],
                        lhsT=outw[:],
                        rhs=hh[:, s : s + 512],
                        start=True,
                        stop=True,
                    )
                ot = otp.tile([1, CH], f32, tag="ot")
                nc.vector.tensor_scalar_add(ot[:], ops2[0:1, :], outb[0:1, 0:1])
                nc.sync.dma_start(o_d.ap()[:, c * CH : (c + 1) * CH], ot[:])

            done = set()
            for l in range(L):
                if l in done:
                    continue
                for c in range(NCH):
                    emit_chunk(l, c)
                    # interleave the tail with the last layer (tail chunk c
                    # needs only layer-29 adds <= c, flushed in (29, c+1))
                    if l == L - 1 and c >= 2:
                        emit_tail(c - 2)
                    if l in EXCH_AFTER and c == NCH - 1:
                        # the staged block only needs chunk 7's add (flushed
                        # inside this chunk's body) — start the collective
                        # before the final flush
                        emit_exchange(l)
                flush(l)
                done.add(l)
                if l in EXCH_AFTER:
                    # wavefront: emit the margin-independent chunks of the
                    # next layer while the collective is in flight, then the
                    # subtract, then the margin-adjacent chunks (depth 1: a
                    # deeper wavefront would pin more pending PSUM buffers
                    # than the pools hold)
                    w0 = l + 1
                    for c in range(NCH - 2):
                        emit_chunk(w0, c)
                    emit_subtract(l)
                    for c in range(NCH - 2, NCH):
                        emit_chunk(w0, c)
                    flush(w0)
                    done.add(w0)

            # ---- tail flush: the interleave covered chunks 0..NCH-3 ----
            emit_tail(NCH - 2)

    nc.compile()
    return nc


# --------------------------------------------------------------------------
# host-side weight folding
# --------------------------------------------------------------------------
def _emb_table():
    steps = np.arange(MAX_STEPS, dtype=np.float32)[:, None]
    dims = np.arange(64, dtype=np.float32)[None, :]
    t = steps * 10.0 ** (dims * 4.0 / 63.0)
    return np.concatenate([np.sin(t), np.cos(t)], axis=1).astype(np.float32)


def _prep_maps(inputs):
    f = lambda a: np.ascontiguousarray(np.asarray(a), dtype=np.float32)
    audio = f(inputs["audio"])          # [B,1,T]
    step = np.asarray(inputs["diffusion_step"]).astype(np.int64)  # [B]
    in_W, in_b = f(inputs["in_W"]), f(inputs["in_b"])
    p1_W, p1_b = f(inputs["p1_W"]), f(inputs["p1_b"])
    p2_W, p2_b = f(inputs["p2_W"]), f(inputs["p2_b"])
    dw_W, dw_b = f(inputs["dw_W"]), f(inputs["dw_b"])
    dp_W, dp_b = f(inputs["dp_W"]), f(inputs["dp_b"])
    op_W, op_b = f(inputs["op_W"]), f(inputs["op_b"])
    sk_W, sk_b = f(inputs["sk_W"]), f(inputs["sk_b"])
    out_W, out_b = f(inputs["out_W"]), f(inputs["out_b"])

    sc = np.float32(2.0) ** (-np.arange(L, dtype=np.float32) / 2)   # 2^(-l/2)
    scu = np.float32(2.0) ** (np.arange(L, dtype=np.float32) / 2)   # 2^(+l/2)

    # gate half computed as tanh(g/2): scale gate output channels by 0.5
    Sg = np.ones((128, 1), np.float32)
    Sg[0:C] = 0.5

    # dilated conv lhsT per parity: mirrored cores (h=1) run on the reversed
    # sequence, so their tap order flips (k -> 2-k)
    w3p = []
    for mir in (False, True):
        w3 = np.zeros((C, L * 3 * 128), np.float32)
        for l in range(L):
            for k in range(3):
                kk = 2 - k if mir else k
                w = dw_W[l, :, :, kk] * sc[l] * Sg        # [128(out), 64(in)]
                w3[:, (l * 3 + k) * 128 : (l * 3 + k + 1) * 128] = w.T
        w3p.append(w3)

    # 1x1 conv lhsT: input is (tanh(g/2)+1)*tanh(f) = 2*yg, so all cols * 0.5;
    # residual cols additionally * 2^(l/2) (deferred sqrt2). Rows duplicated:
    # the device computes opw.T @ (P + tf) with P = tg*tf on rows 0-63 and
    # tf on rows 64-127 of the K=128 rhs.
    opw = np.zeros((128, L * 128), np.float32)
    for l in range(L):
        w = op_W[l, :, :, 0] * 0.5                       # [128(out), 64(in)]
        w[0:C] *= scu[l]
        opw[0:C, l * 128 : (l + 1) * 128] = w.T
        opw[C:128, l * 128 : (l + 1) * 128] = w.T

    # per-layer ACT bias vectors (host-computed; the conditioner MLP is tiny):
    # beff[:,l] = Sg * (wsum_raw[l] @ cond_l + dw_b[l] + 2^(-l/2) * wsum_raw[l] @ Omega_l)
    # with Omega_l = sum_{j<l} 2^(j/2) * op_b[j,:64]; edge variants drop the
    # out-of-window tap. cond_l = dp_W[l] @ silu-MLP(emb[step]) + dp_b[l].
    wsum_raw = dw_W.sum(axis=3)                          # [L,128,64]
    wtl_raw = dw_W[:, :, :, 1:].sum(axis=3)              # drops tap 0 (the -d tap)
    wtr_raw = dw_W[:, :, :, :2].sum(axis=3)              # drops tap 2 (the +d tap)
    table = _emb_table()
    silu = lambda v: v / (1.0 + np.exp(-v))
    beffs = []                                           # per-batch (beff, beffL, beffR)
    for b in range(B):
        t = table[int(step[b])]                          # [128]
        t = silu(p1_W @ t + p1_b)
        t = silu(p2_W @ t + p2_b)
        cond = dp_W @ t + dp_b                           # [L, 64]
        bf = np.zeros((128, L), np.float32)
        bl = np.zeros((128, L), np.float32)
        br = np.zeros((128, L), np.float32)
        omega = np.zeros(C, np.float32)
        for l in range(L):
            bf[:, l] = Sg[:, 0] * (wsum_raw[l] @ (cond[l] + sc[l] * omega) + dw_b[l])
            bl[:, l] = Sg[:, 0] * (wtl_raw[l] @ (cond[l] + sc[l] * omega) + dw_b[l])
            br[:, l] = Sg[:, 0] * (wtr_raw[l] @ (cond[l] + sc[l] * omega) + dw_b[l])
            omega = omega + scu[l] * op_b[l, 0:C]
        beffs.append((bf.astype(np.float32), bl.astype(np.float32), br.astype(np.float32)))

    # tail foldings
    opb_sk_sum = op_b[:, C:].sum(axis=0)                 # [64]
    skw = np.zeros((128, C), np.float32)
    skw[C:128] = (sk_W[:, :, 0] / np.sqrt(np.float32(L))).T
    skb = (sk_b + sk_W[:, :, 0] @ opb_sk_sum / np.sqrt(np.float32(L))).reshape(C, 1)
    outw = out_W[0, :, 0].reshape(C, 1)
    outb = out_b.reshape(1, 1)

    import ml_dtypes

    shared = {
        "opw": opw.astype(ml_dtypes.bfloat16),
        "inw": in_W[:, 0, 0].reshape(1, C),
        "inb": in_b.reshape(C, 1),
        "skw": skw,
        "skb": skb,
        "outw": outw.astype(ml_dtypes.bfloat16),
        "outb": outb,
        "zeros": np.zeros((C, CH), np.float32),
    }

    in_maps = []
    for core in range(8):
        b, h = core // 2, core % 2
        m = dict(shared)
        m["w3"] = w3p[h]
        bf, bl, br = beffs[b]
        m["beff"] = bf
        if h == 0:
            m["aud"] = np.ascontiguousarray(audio[b, 0, 0:E].reshape(1, E))
            m["beffL"], m["beffR"] = bl, br
        else:
            # time-mirrored: reversed audio window; edge variants swap
            m["aud"] = np.ascontiguousarray(audio[b, 0, T - E : T][::-1].reshape(1, E))
            m["beffL"], m["beffR"] = br, bl
        in_maps.append(m)
    return in_maps


def _get_nc():
    if "nc" not in _CACHE:
        _CACHE["nc"] = _build_program()
    return _CACHE["nc"]


def unshard(res_o):
    """res_o: list of 8 per-core 'o' arrays [1, OWN] -> full [B, 1, T]."""
    out = np.zeros((B, 1, T), np.float32)
    for b in range(B):
        out[b, 0, 0:OWN] = res_o[2 * b][0, 0:OWN]
        out[b, 0, OWN:T] = res_o[2 * b + 1][0, 0:OWN][::-1]
    return out


def kernel(**inputs) -> np.ndarray:
    from concourse.bass_utils import run_bass_kernel_spmd

    nc = _get_nc()
    in_maps = _prep_maps(inputs)
    res = run_bass_kernel_spmd(nc, in_maps, core_ids=list(range(8))).results
    return unshard([res[c]["o"] for c in range(8)])


# revision 28
# speedup vs baseline: 1.6050x; 1.6050x over previous
"""DiffWave forward pass on 8 Trainium2 NeuronCores (Bass/Tile).

Sharding: core c -> (batch b = c//2, sequence half h = c%2). Each core computes
its 8192-sample half over a window E = 8192 + 1024: the 1024-column halo is
refreshed twice (after layers 9 and 19, the dilation-cycle boundaries, where
the receptive field per 10-layer block is 1023) by a pairwise inter-core
exchange, instead of the communication-free 3072-column halo a full 30-layer
receptive field would need. Odd cores store their half TIME-MIRRORED (host
reverses their audio window and tap order of the dilated-conv weights, and
swaps the left/right edge-bias corrections), which makes the exchange fully
symmetric SPMD: every core sends local columns [E-2048, E-1024) forward,
and refreshes its margin [E-1024, E) with the partner's block read reversed.
The exchange itself is a pairwise ReduceScatter(add) through DRAM with the
send block duplicated into both slots; the receiver subtracts its own staged
copy to recover the partner's block (exact up to one f32 rounding).

Per-core layout: resident SBUF tensor xs[128, 1024+9216+1024] (float32r):
rows 0-63 = residual trunk x (deferred 1/sqrt(2) scaling folded into weights),
rows 64-127 = skip accumulator. Dilated conv = 3 accumulating float32r matmuls
(K=64, full rate at N=512) per 512-col slice reading shifted views of xs.
Gating runs as ONE packed ACT tanh per 1024-col chunk over all 128 partitions
(sigmoid(g) = (tanh(g/2)+1)/2 with the 0.5 folded into the gate-half weights
and biases), then a DMA partition-shift of the filter half and ONE in-place
DVE bf16 multiply P = tg*tf over the gate rows of sg. The (tg+1)*tf
expansion's "+tf" term is folded into the 1x1 conv instead: that conv runs as
a K=128 matmul whose lhsT carries the op weights duplicated on rows 0-63
(applied to P) and rows 64-127 (applied to tf) — matmul cost is K-independent
so this is free, and it removes the DVE +1 tensor_scalar op. The per-layer
conditioner bias (sum-of-taps dw_W @ cond + dw_b + deferred op-bias
corrections) is applied for free via the ACT bias operand, with edge-corrected
variants on the first/last dilation-width columns. In-layer in-place updates
are deferred by one chunk so neighbouring chunks read pre-update boundary
columns.
"""

import os
import sys

sys.path.insert(0, "/opt/trn_rl_repo")

import numpy as np

import concourse.bacc as bacc
import concourse.mybir as mybir
import concourse.tile as tile
from concourse.ap import AP

f32 = mybir.dt.float32
f32r = mybir.dt.float32r
bf16 = mybir.dt.bfloat16
AF = mybir.ActivationFunctionType
ALU = mybir.AluOpType

C = 64
L = 30
B = 4
T = 16384
MAX_STEPS = 200
OWN = T // 2          # 8192 owned samples per core
PAD = 512             # frozen zero pads (max dilation)
MARGIN = 1024         # halo refreshed at each 10-layer block boundary
E = OWN + MARGIN      # 9216 compute window
WBUF = PAD + E + PAD  # 10240
CH = 1024             # column chunk (2 PSUM banks)
NCH = E // CH         # 9
DILS = [2 ** (i % 10) for i in range(L)]
EXCH_AFTER = (9, 19)  # halo exchange after these layers

_CACHE = {}


def _rev_ap(t, ncols, width, nparts=128):
    """Reversed-column AP over tile t[nparts, ncols], columns [0, width) read
    backwards. Only valid for plain f32/bf16 tiles (f32r is layout-swizzled
    and mis-reads under negative stride)."""
    return AP(tensor=t[:].tensor, offset=width - 1, ap=[[ncols, nparts], [-1, width]])


# --------------------------------------------------------------------------
# device program
# --------------------------------------------------------------------------
def _build_program(dbg=False):
    nc = bacc.Bacc(
        "TRN2",
        target_bir_lowering=False,
        debug=False,
        enable_asserts=False,
        num_devices=8,
    )

    dram = {}

    def din(name, shape, dtype):
        dram[name] = nc.dram_tensor(name, list(shape), dtype, kind="ExternalInput")
        return dram[name]

    din("aud", [1, E], f32r)
    din("w3", [C, L * 3 * 128], f32r)          # dilated conv lhsT per (l, tap)
    din("opw", [128, L * 128], bf16)           # 1x1 conv lhsT per l, rows duplicated
    # per-layer ACT bias vectors (cond + dw_b + deferred op-bias corrections),
    # computed on the host (the conditioner MLP is ~1e-4 of the FLOPs)
    din("beff", [128, L], f32)
    din("beffL", [128, L], f32)                # local-left edge (drops the -d tap)
    din("beffR", [128, L], f32)                # local-right edge (drops the +d tap)
    din("inw", [1, C], f32r)
    din("inb", [C, 1], f32)
    din("skw", [128, C], f32r)                 # rows 64-127 hold sk_W.T/sqrt(30)
    din("skb", [C, 1], f32)
    din("outw", [C, 1], bf16)
    din("outb", [1, 1], f32)
    din("zeros", [C, CH], f32r)
    o_d = nc.dram_tensor("o", [1, OWN], f32, kind="ExternalOutput")
    cc_in = [
        nc.dram_tensor(f"cc_in{i}", [2 * C, MARGIN], f32, kind="Internal")
        for i in range(len(EXCH_AFTER))
    ]
    cc_out = [
        nc.dram_tensor(f"cc_out{i}", [C, MARGIN], f32, kind="Internal")
        for i in range(len(EXCH_AFTER))
    ]

    with tile.TileContext(nc) as tc:
        import contextlib

        ctx = contextlib.ExitStack()
        with ctx:
            const = ctx.enter_context(tc.tile_pool(name="const", bufs=1))
            sgp = ctx.enter_context(tc.tile_pool(name="sgp", bufs=4))
            sgfp = ctx.enter_context(tc.tile_pool(name="sgfp", bufs=4))
            hhp = ctx.enter_context(tc.tile_pool(name="hhp", bufs=2))
            otp = ctx.enter_context(tc.tile_pool(name="otp", bufs=2))
            exp = ctx.enter_context(tc.tile_pool(name="exp", bufs=2))
            dil_ps = ctx.enter_context(tc.tile_pool(name="dil_ps", bufs=2, space="PSUM"))
            op_ps = ctx.enter_context(tc.tile_pool(name="op_ps", bufs=2, space="PSUM"))

            # ---- resident state + weights ----
            xs = const.tile([128, WBUF], f32r)
            w3 = const.tile([C, L * 3 * 128], f32r)
            opw = const.tile([128, L * 128], bf16)
            beff = const.tile([128, L], f32)
            beffL = const.tile([128, L], f32)
            beffR = const.tile([128, L], f32)
            inw = const.tile([1, C], f32r)
            inb = const.tile([C, 1], f32)
            skw = const.tile([128, C], f32r)
            skb = const.tile([C, 1], f32)
            outw = const.tile([C, 1], bf16)
            outb = const.tile([1, 1], f32)

            nc.sync.dma_start(inw[:], dram["inw"].ap())
            nc.sync.dma_start(inb[:], dram["inb"].ap())
            nc.sync.dma_start(beff[:], dram["beff"].ap())
            nc.sync.dma_start(beffL[:], dram["beffL"].ap())
            nc.sync.dma_start(beffR[:], dram["beffR"].ap())

            # ---- zero pads and skip accumulator (DMA: memset-f32r fails codegen) ----
            nc.sync.dma_start(xs[0:C, 0:PAD], dram["zeros"].ap()[:, 0:PAD])
            nc.sync.dma_start(xs[0:C, PAD + E : WBUF], dram["zeros"].ap()[:, 0:PAD])
            for c in range(WBUF // CH):
                nc.sync.dma_start(
                    xs[C:128, c * CH : (c + 1) * CH], dram["zeros"].ap()
                )

            with tc.tile_pool(name="audp", bufs=2) as audp:
                # ---- input conv: x0 = relu(in_W * audio + in_b); the big
                # weight DMAs are issued after it so the audio chunks reach
                # the DMA engines first and compute starts immediately ----
                for c in range(NCH):
                    at = audp.tile([1, CH], f32r, tag="aud")
                    nc.sync.dma_start(at[:], dram["aud"].ap()[:, c * CH : (c + 1) * CH])
                    x0 = dil_ps.tile([128, CH], f32, tag="dil")
                    for s in (0, 512):
                        nc.tensor.matmul(
                            x0[0:C, s : s + 512],
                            lhsT=inw[:],
                            rhs=at[:, s : s + 512],
                            start=True,
                            stop=True,
                        )
                    nc.scalar.activation(
                        xs[0:C, PAD + c * CH : PAD + (c + 1) * CH],
                        x0[0:C, :],
                        AF.Relu,
                        bias=inb[:, 0:1],
                    )
                    if c == 0:
                        nc.sync.dma_start(w3[:], dram["w3"].ap())
                        nc.sync.dma_start(opw[:], dram["opw"].ap())
                    if c == 2:
                        nc.sync.dma_start(skw[:], dram["skw"].ap())
                        nc.sync.dma_start(skb[:], dram["skb"].ap())
                        nc.sync.dma_start(outw[:], dram["outw"].ap())
                        nc.sync.dma_start(outb[:], dram["outb"].ap())

            # ---- 30 residual layers ----
            # pend[l]: chunk c+1's tap-0 matmuls read the last d columns of
            # chunk c, so chunk c's in-place update is emitted only after chunk
            # c+1's dilated-conv reads (one-chunk lag; Tile orders by program
            # order). The emission schedule runs chunks in a wavefront around
            # each halo exchange so the engines' in-order queues never park on
            # a collective-dependent instruction (head-of-line blocking).
            pend = {}
            ex_state = {}

            def flush(l):
                if pend.get(l) is not None:
                    pcol, pop = pend[l]
                    nc.vector.tensor_add(
                        xs[:, pcol : pcol + CH], xs[:, pcol : pcol + CH], pop[:]
                    )
                    pend[l] = None

            def emit_chunk(l, c, share_psum=False):
                d = DILS[l]
                col = PAD + c * CH
                dil = dil_ps.tile([128, CH], f32, tag="dil")
                for k in range(3):
                    off = (k - 1) * d
                    for s in (0, 512):
                        nc.tensor.matmul(
                            dil[:, s : s + 512],
                            lhsT=w3[:, (l * 3 + k) * 128 : (l * 3 + k + 1) * 128],
                            rhs=xs[0:C, col + off + s : col + off + s + 512],
                            start=(k == 0),
                            stop=(k == 2),
                        )
                sg = sgp.tile([128, CH], bf16, tag="sg")
                # (column range, bias) pieces: sequence-edge columns use the
                # tap-dropped bias (reference zero-pads x+cond, so the
                # missing tap must not contribute cond/omega via the bias)
                if c == 0:
                    pieces = [(0, d, beffL), (d, CH, beff)]
                elif c == NCH - 1:
                    pieces = [(0, CH - d, beff), (CH - d, CH, beffR)]
                else:
                    pieces = [(0, CH, beff)]
                # single packed tanh: rows 0-63 hold tanh(g/2) (gate half
                # pre-scaled 0.5 in weights; sigmoid(g) = (tanh(g/2)+1)/2),
                # rows 64-127 hold tanh(f)
                for lo, hi, bv in pieces:
                    nc.scalar.activation(
                        sg[:, lo:hi], dil[:, lo:hi], AF.Tanh,
                        bias=bv[:, l : l + 1],
                    )
                sgf = sgfp.tile([C, CH], bf16, tag="sgf")
                nc.sync.dma_start(sgf[:], sg[C:128, :])
                # P = tg*tf in place over the gate rows; the op conv below
                # contracts K=128 over [P; tf] with duplicated op weights,
                # realizing (tg+1)*tf without a separate +1 op
                nc.vector.tensor_mul(sg[0:C, :], sg[0:C, :], sgf[:])
                flush(l)
                # share_psum: write the op conv over this chunk's dil tile
                # (tanh consumed it) so a wavefront layer pins no op_ps buffer
                op = dil if share_psum else op_ps.tile([128, CH], f32, tag="op")
                for s in (0, 512):
                    nc.tensor.matmul(
                        op[:, s : s + 512],
                        lhsT=opw[:, l * 128 : (l + 1) * 128],
                        rhs=sg[:, s : s + 512],
                        start=True,
                        stop=True,
                    )
                pend[l] = (col, op)

            def emit_exchange(l):
                # trunk rows only (margin skip is never consumed), f32; all
                # DMAs on the idle gpsimd (SWDGE) queue so the in-order
                # SP/DVE streams never block behind the collective
                ex = EXCH_AFTER.index(l)
                send = xs[0:C, PAD + E - 2 * MARGIN : PAD + E - MARGIN]
                nc.gpsimd.dma_start(cc_in[ex].ap()[0:C], send)
                nc.gpsimd.dma_start(cc_in[ex].ap()[C : 2 * C], send)
                nc.gpsimd.collective_compute(
                    "ReduceScatter", ALU.add,
                    replica_groups=[[0, 1], [2, 3], [4, 5], [6, 7]],
                    ins=[cc_in[ex].ap()], outs=[cc_out[ex].ap()],
                )
                rt = exp.tile([C, MARGIN], f32, tag="rt")
                nc.gpsimd.dma_start(rt[:], cc_out[ex].ap())
                # forward f32r->f32 copy of the sent block, consumed reversed
                # by the subtract (f32r cannot be read with negative stride);
                # runs any time while the collective is in flight
                stage = exp.tile([C, MARGIN], f32, tag="stage")
                nc.vector.tensor_copy(stage[:], send)
                ex_state[l] = (rt, stage)

            def emit_subtract(l):
                # margin <- reverse(partner block) = reverse(rt - stage);
                # both reads reversed, write forward into the f32r trunk
                rt, stage = ex_state.pop(l)
                nc.vector.tensor_tensor(
                    xs[0:C, PAD + E - MARGIN : PAD + E],
                    _rev_ap(rt, MARGIN, MARGIN, C),
                    _rev_ap(stage, MARGIN, MARGIN, C),
                    ALU.subtract,
                )

            def emit_tail(c):
                # skip head + output conv over one owned chunk (the margin
                # chunk's output is discarded by the host)
                col = PAD + c * CH
                hps = dil_ps.tile([128, CH], f32, tag="dil")
                for s in (0, 512):
                    nc.tensor.matmul(
                        hps[0:C, s : s + 512],
                        lhsT=skw[C:128, :],
                        rhs=xs[C:128, col + s : col + s + 512],
                        start=True,
                        stop=True,
                        tile_position=(64, 0),
                    )
                hh = hhp.tile([C, CH], bf16, tag="hh")
                nc.scalar.activation(hh[:], hps[0:C, :], AF.Relu, bias=skb[:, 0:1])
                ops2 = op_ps.tile([128, CH], f32, tag="op")
                for s in (0, 512):
                    nc.tensor.matmul(
                        ops2[0:1, s : s + 512],
                        lhsT=outw[:],
                        rhs=hh[:, s : s + 512],
                        start=True,
                        stop=True,
                    )
                ot = otp.tile([1, CH], f32, tag="ot")
                nc.vector.tensor_scalar_add(ot[:], ops2[0:1, :], outb[0:1, 0:1])
                nc.sync.dma_start(o_d.ap()[:, c * CH : (c + 1) * CH], ot[:])

            done = set()
            for l in range(L):
                if l in done:
                    continue
                for c in range(NCH):
                    emit_chunk(l, c)
                    # interleave the tail with the last layer (tail chunk c
                    # needs only layer-29 adds <= c, flushed in (29, c+1))
                    if l == L - 1 and c >= 2:
                        emit_tail(c - 2)
                    if l in EXCH_AFTER and c == NCH - 1:
                        # the staged block only needs chunk 7's add (flushed
                        # inside this chunk's body) — start the collective
                        # before the final flush
                        emit_exchange(l)
                flush(l)
                done.add(l)
                if l in EXCH_AFTER:
                    # wavefront: emit the margin-independent chunks of the
                    # next two layers while the collective is in flight, then
                    # the subtract, then the margin-adjacent chunks. The
                    # second layer runs with share_psum so only layer w0's
                    # pending op_ps buffer is pinned across the gap.
                    w0 = l + 1
                    for c in range(NCH - 2):
                        emit_chunk(w0, c)
                    for c in range(NCH - 4):
                        emit_chunk(w0 + 1, c, share_psum=True)
                    emit_subtract(l)
                    for c in range(NCH - 2, NCH):
                        emit_chunk(w0, c)
                    flush(w0)
                    done.add(w0)
                    for c in range(NCH - 4, NCH):
                        emit_chunk(w0 + 1, c)
                    flush(w0 + 1)
                    done.add(w0 + 1)

            # ---- tail flush: the interleave covered chunks 0..NCH-3 ----
            emit_tail(NCH - 2)

    nc.compile()
    return nc


# --------------------------------------------------------------------------
# host-side weight folding
# --------------------------------------------------------------------------
def _emb_table():
    steps = np.arange(MAX_STEPS, dtype=np.float32)[:, None]
    dims = np.arange(64, dtype=np.float32)[None, :]
    t = steps * 10.0 ** (dims * 4.0 / 63.0)
    return np.concatenate([np.sin(t), np.cos(t)], axis=1).astype(np.float32)


def _prep_maps(inputs):
    f = lambda a: np.ascontiguousarray(np.asarray(a), dtype=np.float32)
    audio = f(inputs["audio"])          # [B,1,T]
    step = np.asarray(inputs["diffusion_step"]).astype(np.int64)  # [B]
    in_W, in_b = f(inputs["in_W"]), f(inputs["in_b"])
    p1_W, p1_b = f(inputs["p1_W"]), f(inputs["p1_b"])
    p2_W, p2_b = f(inputs["p2_W"]), f(inputs["p2_b"])
    dw_W, dw_b = f(inputs["dw_W"]), f(inputs["dw_b"])
    dp_W, dp_b = f(inputs["dp_W"]), f(inputs["dp_b"])
    op_W, op_b = f(inputs["op_W"]), f(inputs["op_b"])
    sk_W, sk_b = f(inputs["sk_W"]), f(inputs["sk_b"])
    out_W, out_b = f(inputs["out_W"]), f(inputs["out_b"])

    sc = np.float32(2.0) ** (-np.arange(L, dtype=np.float32) / 2)   # 2^(-l/2)
    scu = np.float32(2.0) ** (np.arange(L, dtype=np.float32) / 2)   # 2^(+l/2)

    # gate half computed as tanh(g/2): scale gate output channels by 0.5
    Sg = np.ones((128, 1), np.float32)
    Sg[0:C] = 0.5

    # dilated conv lhsT per parity: mirrored cores (h=1) run on the reversed
    # sequence, so their tap order flips (k -> 2-k)
    w3p = []
    for mir in (False, True):
        w3 = np.zeros((C, L * 3 * 128), np.float32)
        for l in range(L):
            for k in range(3):
                kk = 2 - k if mir else k
                w = dw_W[l, :, :, kk] * sc[l] * Sg        # [128(out), 64(in)]
                w3[:, (l * 3 + k) * 128 : (l * 3 + k + 1) * 128] = w.T
        w3p.append(w3)

    # 1x1 conv lhsT: input is (tanh(g/2)+1)*tanh(f) = 2*yg, so all cols * 0.5;
    # residual cols additionally * 2^(l/2) (deferred sqrt2). Rows duplicated:
    # the device computes opw.T @ (P + tf) with P = tg*tf on rows 0-63 and
    # tf on rows 64-127 of the K=128 rhs.
    opw = np.zeros((128, L * 128), np.float32)
    for l in range(L):
        w = op_W[l, :, :, 0] * 0.5                       # [128(out), 64(in)]
        w[0:C] *= scu[l]
        opw[0:C, l * 128 : (l + 1) * 128] = w.T
        opw[C:128, l * 128 : (l + 1) * 128] = w.T

    # per-layer ACT bias vectors (host-computed; the conditioner MLP is tiny):
    # beff[:,l] = Sg * (wsum_raw[l] @ cond_l + dw_b[l] + 2^(-l/2) * wsum_raw[l] @ Omega_l)
    # with Omega_l = sum_{j<l} 2^(j/2) * op_b[j,:64]; edge variants drop the
    # out-of-window tap. cond_l = dp_W[l] @ silu-MLP(emb[step]) + dp_b[l].
    wsum_raw = dw_W.sum(axis=3)                          # [L,128,64]
    wtl_raw = dw_W[:, :, :, 1:].sum(axis=3)              # drops tap 0 (the -d tap)
    wtr_raw = dw_W[:, :, :, :2].sum(axis=3)              # drops tap 2 (the +d tap)
    table = _emb_table()
    silu = lambda v: v / (1.0 + np.exp(-v))
    beffs = []                                           # per-batch (beff, beffL, beffR)
    for b in range(B):
        t = table[int(step[b])]                          # [128]
        t = silu(p1_W @ t + p1_b)
        t = silu(p2_W @ t + p2_b)
        cond = dp_W @ t + dp_b                           # [L, 64]
        bf = np.zeros((128, L), np.float32)
        bl = np.zeros((128, L), np.float32)
        br = np.zeros((128, L), np.float32)
        omega = np.zeros(C, np.float32)
        for l in range(L):
            bf[:, l] = Sg[:, 0] * (wsum_raw[l] @ (cond[l] + sc[l] * omega) + dw_b[l])
            bl[:, l] = Sg[:, 0] * (wtl_raw[l] @ (cond[l] + sc[l] * omega) + dw_b[l])
            br[:, l] = Sg[:, 0] * (wtr_raw[l] @ (cond[l] + sc[l] * omega) + dw_b[l])
            omega = omega + scu[l] * op_b[l, 0:C]
        beffs.append((bf.astype(np.float32), bl.astype(np.float32), br.astype(np.float32)))

    # tail foldings
    opb_sk_sum = op_b[:, C:].sum(axis=0)                 # [64]
    skw = np.zeros((128, C), np.float32)
    skw[C:128] = (sk_W[:, :, 0] / np.sqrt(np.float32(L))).T
    skb = (sk_b + sk_W[:, :, 0] @ opb_sk_sum / np.sqrt(np.float32(L))).reshape(C, 1)
    outw = out_W[0, :, 0].reshape(C, 1)
    outb = out_b.reshape(1, 1)

    import ml_dtypes

    shared = {
        "opw": opw.astype(ml_dtypes.bfloat16),
        "inw": in_W[:, 0, 0].reshape(1, C),
        "inb": in_b.reshape(C, 1),
        "skw": skw,
        "skb": skb,
        "outw": outw.astype(ml_dtypes.bfloat16),
        "outb": outb,
        "zeros": np.zeros((C, CH), np.float32),
    }

    in_maps = []
    for core in range(8):
        b, h = core // 2, core % 2
        m = dict(shared)
        m["w3"] = w3p[h]
        bf, bl, br = beffs[b]
        m["beff"] = bf
        if h == 0:
            m["aud"] = np.ascontiguousarray(audio[b, 0, 0:E].reshape(1, E))
            m["beffL"], m["beffR"] = bl, br
        else:
            # time-mirrored: reversed audio window; edge variants swap
            m["aud"] = np.ascontiguousarray(audio[b, 0, T - E : T][::-1].reshape(1, E))
            m["beffL"], m["beffR"] = br, bl
        in_maps.append(m)
    return in_maps


def _get_nc():
    if "nc" not in _CACHE:
        _CACHE["nc"] = _build_program()
    return _CACHE["nc"]


def unshard(res_o):
    """res_o: list of 8 per-core 'o' arrays [1, OWN] -> full [B, 1, T]."""
    out = np.zeros((B, 1, T), np.float32)
    for b in range(B):
        out[b, 0, 0:OWN] = res_o[2 * b][0, 0:OWN]
        out[b, 0, OWN:T] = res_o[2 * b + 1][0, 0:OWN][::-1]
    return out


def kernel(**inputs) -> np.ndarray:
    from concourse.bass_utils import run_bass_kernel_spmd

    nc = _get_nc()
    in_maps = _prep_maps(inputs)
    res = run_bass_kernel_spmd(nc, in_maps, core_ids=list(range(8))).results
    return unshard([res[c]["o"] for c in range(8)])
